# revision 6
# baseline (speedup 1.0000x reference)
"""Trainium2 Bass kernel for block-diagonal sparse attention (8 NeuronCores SPMD).

Problem: nn_AttentionHead (N=4096, DIM_IN=512, DQ=DK=128, 16 graphs of 256 nodes).
  q = x@Wq.T+bq; k = x@Wk.T+bk; v = x@Wv.T+bv
  a = where(block, qk/sqrt(dq), 0) + b + c; masked-softmax over block-diagonal
  out = (softmax(a)*keep) @ v

Key structural facts exploited:
  - Everything off the block diagonal is masked to -1e6 -> exp underflows to 0,
    so only the 16 diagonal 256x256 tiles of b/c/sparse_mask matter. The host
    slices exactly those blocks, cutting HBM traffic from ~200MB to ~3.5MB/core.
  - Graphs are independent -> rows shard 2-graphs-per-core across 8 cores with
    zero cross-core communication (weights replicated).
  - softmax(a)[r] = exp(a[r])/sum(exp(a[r])): |a| <~ 15 so no max-subtraction is
    needed in f32; masked entries get exp(a-100) which underflows vs kept terms.
  - The denominator is obtained for free by appending a ones-column to v in the
    PV matmul (column 128 of the PSUM accumulates sum_j e[r,j]).

Layout: all projections are computed transposed (d on partitions) straight from
x.T, scores are computed transposed (j on partitions) so the probability matrix
comes out in exactly the layout the PV matmul needs as its stationary operand
(no on-chip transpose of e). Only v needs a PE transpose back to natural layout.
"""

import math

import numpy as np

import concourse.bass as bass
import concourse.mybir as mybir
import concourse.tile as tile
from concourse import bacc
from concourse.bass_utils import run_bass_kernel_spmd
from concourse.masks import make_identity

# -------- problem constants (hardcoded per spec) --------
N = 4096
DIN = 512
DQ = 128           # == DK
NG = 16            # number of graphs
G = N // NG        # 256 nodes per graph
NCORES = 8
RPC = N // NCORES  # 512 rows per core
GPC = NG // NCORES  # 2 graphs per core
NT = RPC // 128    # 4 row-tiles of 128 per core
KO = DIN // 128    # 4 contraction tiles for the projections
VA = DQ + 1        # v augmented with a ones column (denominator trick)
SCALE = 1.0 / math.sqrt(DQ)
SENT = 100.0       # mask sentinel; exp bias of -SENT cancels it for kept entries

F32 = mybir.dt.float32
F32R = mybir.dt.float32r
BF16 = mybir.dt.bfloat16
I32 = mybir.dt.int32

ACT = mybir.ActivationFunctionType
ALU = mybir.AluOpType

_CACHE: dict = {}


def build_nc() -> bass.Bass:
    """Build the per-core Bass graph (identical on all 8 cores)."""
    nc = bacc.Bacc(
        "TRN2",
        target_bir_lowering=False,
        debug=False,
        enable_asserts=False,
        num_devices=NCORES,
    )
    xT_d = nc.dram_tensor("xT", [DIN, RPC], F32, kind="ExternalInput").ap()
    w_d = nc.dram_tensor("w", [3, DIN, DQ], F32, kind="ExternalInput").ap()
    bia_d = nc.dram_tensor("bias", [DQ, 3], F32, kind="ExternalInput").ap()
    bc_d = nc.dram_tensor("bcT", [2, RPC, G], F32, kind="ExternalInput").ap()
    md_d = nc.dram_tensor("mdT", [RPC, G], I32, kind="ExternalInput").ap()
    out_d = nc.dram_tensor("out", [RPC, DQ], F32, kind="ExternalOutput").ap()

    with tile.TileContext(nc) as tc:
        with (
            tc.tile_pool(name="const", bufs=1) as cpool,
            tc.tile_pool(name="work", bufs=3) as wpool,
            tc.tile_pool(name="et", bufs=4) as epool,
            tc.tile_pool(name="ps_proj", bufs=2, space="PSUM") as pp,
            tc.tile_pool(name="ps_tr", bufs=2, space="PSUM") as pt,
            tc.tile_pool(name="ps_s", bufs=2, space="PSUM") as ps,
            tc.tile_pool(name="ps_o", bufs=2, space="PSUM") as po,
        ):
            # ---- input DMAs (few, large, contiguous >=512B segments) ----
            # gpsimd (SWDGE) DMAs cast f32 -> bf16 in flight
            xT = cpool.tile([128, KO, RPC], BF16)  # [din%128, din//128, r]
            nc.gpsimd.dma_start(out=xT[:], in_=xT_d.rearrange("(o p) r -> p o r", p=128))
            w = cpool.tile([128, 3, KO, DQ], BF16)  # [din%128, qkv, din//128, d]
            nc.gpsimd.dma_start(out=w[:], in_=w_d.rearrange("s (o p) d -> p s o d", p=128))
            bia = cpool.tile([128, 3], F32)  # [d, qkv]
            nc.sync.dma_start(bia[:], bia_d)
            bc = cpool.tile([128, 2, NT, G], F32)  # [j%128, b|c, j//128, r]
            nc.sync.dma_start(bc[:], bc_d.rearrange("s (t p) r -> p s t r", p=128))
            md = cpool.tile([128, NT, G], I32)
            nc.sync.dma_start(md[:], md_d.rearrange("(t p) r -> p t r", p=128))

            ident = cpool.tile([128, 128], BF16)
            make_identity(nc, ident[:])

            # q bias is folded together with the 1/sqrt(dq) scale
            bqs = cpool.tile([128, 1], F32)
            nc.vector.tensor_scalar_mul(bqs[:], bia[:, 0:1], SCALE)

            # per-partition constant -SENT used as the exp bias
            negs = cpool.tile([128, 1], F32)
            nc.vector.memset(negs[:], -SENT)

            # ---- projections, transposed: pT[d, r] = (x @ W_s.T).T ----
            def proj(s):
                p = pp.tile([128, RPC], F32, tag="proj")
                for ko in range(KO):
                    nc.tensor.matmul(
                        p[:],
                        lhsT=w[:, s, ko, :],
                        rhs=xT[:, ko, :],
                        start=(ko == 0),
                        stop=(ko == KO - 1),
                    )
                return p

            qT = cpool.tile([128, RPC], BF16)
            pq = proj(0)
            nc.scalar.activation(qT[:], pq[:], ACT.Identity, bias=bqs[:], scale=SCALE)
            kT = cpool.tile([128, RPC], BF16)
            pk = proj(1)
            nc.scalar.activation(kT[:], pk[:], ACT.Identity, bias=bia[:, 1:2])
            vT = cpool.tile([128, RPC], BF16)
            pv = proj(2)
            nc.scalar.activation(vT[:], pv[:], ACT.Identity, bias=bia[:, 2:3])

            # ---- v back to natural layout (bf16), ones column appended ----
            vna = cpool.tile([128, NT, VA], BF16)  # [j%128, j//128, d | 1]
            nc.vector.memset(vna[:], 1.0)
            for jt in range(NT):
                trp = pt.tile([128, 128], BF16, tag="tr")
                nc.tensor.transpose(trp[:], vT[:, jt * 128:(jt + 1) * 128], ident[:])
                nc.vector.tensor_copy(out=vna[:, jt, 0:DQ], in_=trp[:])

            # ---- scores (transposed), bias+mask, exp ----
            ets = []
            for t in range(NT):
                g, jb = divmod(t, 2)
                sp = ps.tile([128, G], F32, tag="s")
                nc.tensor.matmul(
                    sp[:],
                    lhsT=kT[:, g * G + jb * 128: g * G + jb * 128 + 128],
                    rhs=qT[:, g * G:(g + 1) * G],
                    start=True,
                    stop=True,
                )
                a = wpool.tile([128, G], F32, tag="a")
                nc.vector.tensor_tensor(a[:], sp[:], bc[:, 0, t, :], ALU.add)
                nc.vector.tensor_tensor(a[:], a[:], bc[:, 1, t, :], ALU.add)
                # a += SENT * mask  (kept entries raised by SENT, masked stay)
                nc.vector.scalar_tensor_tensor(
                    a[:], md[:, t, :], SENT, a[:], op0=ALU.mult, op1=ALU.add
                )
                # exp(a - SENT): kept -> exp(orig), masked -> exp(orig-100) ~ 0
                et = epool.tile([128, G], BF16, tag="et")
                nc.scalar.activation(et[:], a[:], ACT.Exp, bias=negs[:])
                ets.append(et)

            # ---- PV matmul (+denominator via ones column), normalize ----
            out_sb = cpool.tile([128, NT, DQ], F32)
            for g in range(GPC):
                for rb in range(2):
                    t = 2 * g + rb
                    op = po.tile([128, VA], F32, tag="o")
                    for jb in range(2):
                        nc.tensor.matmul(
                            op[:],
                            lhsT=ets[2 * g + jb][:, rb * 128:(rb + 1) * 128],
                            rhs=vna[:, 2 * g + jb, :],
                            start=(jb == 0),
                            stop=(jb == 1),
                        )
                    rec = wpool.tile([128, 1], F32, tag="rec")
                    nc.vector.reciprocal(rec[:], op[:, DQ:VA])
                    nc.scalar.activation(
                        out_sb[:, t, :], op[:, 0:DQ], ACT.Copy, scale=rec[:]
                    )

            nc.sync.dma_start(out_d.rearrange("(t p) d -> p t d", p=128), out_sb[:])
    nc.compile()
    return nc


def get_nc() -> bass.Bass:
    if "nc" not in _CACHE:
        _CACHE["nc"] = build_nc()
    return _CACHE["nc"]


def make_in_maps(x, b, c, ptr, sparse_mask, Wq, bq, Wk, bk, Wv, bv):
    """Host-side sharding: slice the block-diagonal and transpose per layout."""
    x = np.asarray(x, dtype=np.float32)
    b = np.asarray(b, dtype=np.float32)
    c = np.asarray(c, dtype=np.float32)
    ptr = np.asarray(ptr)
    sparse_mask = np.asarray(sparse_mask, dtype=np.int32)
    w = np.ascontiguousarray(
        np.stack([np.asarray(Wq).T, np.asarray(Wk).T, np.asarray(Wv).T])
    ).astype(np.float32)  # [3, DIN, DQ]
    bias = np.ascontiguousarray(
        np.stack([np.asarray(bq), np.asarray(bk), np.asarray(bv)], axis=1)
    ).astype(np.float32)  # [DQ, 3]

    assert np.array_equal(
        np.asarray(ptr).ravel(), np.arange(NG + 1) * G
    ), "kernel compiled for uniform 256-node graphs"

    in_maps = []
    for i in range(NCORES):
        lo = i * RPC
        xT = np.ascontiguousarray(x[lo:lo + RPC].T)  # [DIN, RPC]
        bds, cds, mds = [], [], []
        for gl in range(GPC):
            blk = slice(lo + gl * G, lo + (gl + 1) * G)
            bds.append(b[blk, blk].T)
            cds.append(c[blk, blk].T)
            mds.append(sparse_mask[blk, blk].T)
        bcT = np.ascontiguousarray(
            np.stack([np.concatenate(bds, 0), np.concatenate(cds, 0)])
        ).astype(np.float32)  # [2, RPC, G]
        mdT = np.ascontiguousarray(np.concatenate(mds, 0)).astype(np.int32)
        in_maps.append(
            {"xT": xT, "w": w, "bias": bias, "bcT": bcT, "mdT": mdT}
        )
    return in_maps


def run(inputs: dict, trace: bool = False):
    """Run on all 8 cores; returns (full_output, BassKernelResults)."""
    nc = get_nc()
    in_maps = make_in_maps(**inputs)
    res = run_bass_kernel_spmd(
        nc, in_maps, core_ids=list(range(NCORES)), trace=trace
    )
    out = np.concatenate([r["out"] for r in res.results], axis=0)
    return out.astype(np.float32), res


def kernel(**inputs) -> np.ndarray:
    out, _ = run(inputs, trace=False)
    return out


# revision 8
# speedup vs baseline: 1.2522x; 1.2522x over previous
"""Trainium2 Bass kernel for block-diagonal sparse attention (8 NeuronCores SPMD).

Problem: nn_AttentionHead (N=4096, DIM_IN=512, DQ=DK=128, 16 graphs of 256 nodes).
  q = x@Wq.T+bq; k = x@Wk.T+bk; v = x@Wv.T+bv
  a = where(block, qk/sqrt(dq), 0) + b + c; masked-softmax over block-diagonal
  out = (softmax(a)*keep) @ v

Key structural facts exploited:
  - Everything off the block diagonal is masked to -1e6 -> exp underflows to 0,
    so only the 16 diagonal 256x256 tiles of b/c/sparse_mask matter. The host
    slices exactly those blocks, cutting HBM traffic from ~200MB to ~3.5MB/core.
  - Graphs are independent -> rows shard 2-graphs-per-core across 8 cores with
    zero cross-core communication (weights replicated).
  - softmax(a)[r] = exp(a[r])/sum(exp(a[r])): |a| <~ 15 so no max-subtraction is
    needed in f32; masked entries get exp(a-100) which underflows vs kept terms.
  - The denominator is obtained for free by appending a ones-column to v in the
    PV matmul (column 128 of the PSUM accumulates sum_j e[r,j]).

Layout: q/k projections are computed transposed (d on partitions) straight from
x.T; scores are computed transposed (j on partitions) so the probability matrix
comes out in exactly the layout the PV matmul needs as its stationary operand
(no on-chip transpose of e). v is computed directly in natural layout (rows on
partitions) using x.T slices as the stationary operand; its bias lands via a
rank-1 (K=1) ones x bias^T matmul into the same PSUM accumulation group.

All DMAs ride the sync HWDGE ring (fast path); f32 -> bf16 casts happen on-chip
on DVE/ACT (the gpsimd SWDGE casting-DMA path measured ~4x slower end-to-end).
"""

import math

import numpy as np

import concourse.bass as bass
import concourse.mybir as mybir
import concourse.tile as tile
from concourse import bacc
from concourse.bass_utils import run_bass_kernel_spmd

# -------- problem constants (hardcoded per spec) --------
N = 4096
DIN = 512
DQ = 128           # == DK
NG = 16            # number of graphs
G = N // NG        # 256 nodes per graph
NCORES = 8
RPC = N // NCORES  # 512 rows per core
GPC = NG // NCORES  # 2 graphs per core
NT = RPC // 128    # 4 row-tiles of 128 per core
KO = DIN // 128    # 4 contraction tiles for the projections
VA = DQ + 1        # v augmented with a ones column (denominator trick)
SCALE = 1.0 / math.sqrt(DQ)
SENT = 100.0       # additive mask sentinel; exp bias of -SENT cancels it

F32 = mybir.dt.float32
BF16 = mybir.dt.bfloat16
I32 = mybir.dt.int32

ACT = mybir.ActivationFunctionType
ALU = mybir.AluOpType

_CACHE: dict = {}


def build_nc() -> bass.Bass:
    """Build the per-core Bass graph (identical on all 8 cores)."""
    nc = bacc.Bacc(
        "TRN2",
        target_bir_lowering=False,
        debug=False,
        enable_asserts=False,
        num_devices=NCORES,
    )
    xT_d = nc.dram_tensor("xT", [DIN, RPC], F32, kind="ExternalInput").ap()
    w_d = nc.dram_tensor("w", [3, DIN, DQ], F32, kind="ExternalInput").ap()
    bia_d = nc.dram_tensor("bias", [DQ, 3], F32, kind="ExternalInput").ap()
    biar_d = nc.dram_tensor("biasr", [1, 3, DQ], F32, kind="ExternalInput").ap()
    bc_d = nc.dram_tensor("bcT", [2, RPC, G], F32, kind="ExternalInput").ap()
    md_d = nc.dram_tensor("mdT", [RPC, G], I32, kind="ExternalInput").ap()
    out_d = nc.dram_tensor("out", [RPC, DQ], F32, kind="ExternalOutput").ap()

    with tile.TileContext(nc) as tc:
        with (
            tc.tile_pool(name="const", bufs=1) as cpool,
            tc.tile_pool(name="work", bufs=3) as wpool,
            tc.tile_pool(name="et", bufs=4) as epool,
            tc.tile_pool(name="ps_proj", bufs=2, space="PSUM") as pp,
            tc.tile_pool(name="ps_v", bufs=2, space="PSUM") as pvp,
            tc.tile_pool(name="ps_s", bufs=2, space="PSUM") as ps,
            tc.tile_pool(name="ps_o", bufs=2, space="PSUM") as po,
        ):
            # ---- input DMAs, all on the sync HWDGE ring, critical-path first
            bia = cpool.tile([128, 3], F32)  # [d, qkv]
            nc.sync.dma_start(bia[:], bia_d)
            biar = cpool.tile([1, 3, DQ], F32)  # row layout for the v bias
            nc.sync.dma_start(biar[:], biar_d)
            xT_f = cpool.tile([128, KO, RPC], F32)  # [din%128, din//128, r]
            nc.sync.dma_start(xT_f[:], xT_d.rearrange("(o p) r -> p o r", p=128))
            w_f = cpool.tile([128, 3, KO, DQ], F32)  # [din%128, qkv, din//128, d]
            nc.sync.dma_start(w_f[:], w_d.rearrange("s (o p) d -> p s o d", p=128))
            bc = cpool.tile([128, 2, NT, G], F32)  # [j%128, b|c, j//128, r]
            nc.sync.dma_start(bc[:], bc_d.rearrange("s (t p) r -> p s t r", p=128))
            md = cpool.tile([128, NT, G], I32)
            nc.sync.dma_start(md[:], md_d.rearrange("(t p) r -> p t r", p=128))

            # ---- on-chip f32 -> bf16 casts, split across DVE and ACT ----
            xT = cpool.tile([128, KO, RPC], BF16)
            for ko in range(KO):
                if ko % 2 == 0:
                    nc.vector.tensor_copy(out=xT[:, ko, :], in_=xT_f[:, ko, :])
                else:
                    nc.scalar.copy(xT[:, ko, :], xT_f[:, ko, :])
            w = cpool.tile([128, 3, KO, DQ], BF16)
            for s in range(3):
                if s % 2 == 0:
                    nc.scalar.copy(w[:, s, :, :], w_f[:, s, :, :])
                else:
                    nc.vector.tensor_copy(out=w[:, s, :, :], in_=w_f[:, s, :, :])
            biar_b = cpool.tile([1, 3, DQ], BF16)
            nc.vector.tensor_copy(out=biar_b[:], in_=biar[:])

            # small constants
            bqs = cpool.tile([128, 1], F32)  # bq / sqrt(dq)
            nc.vector.tensor_scalar_mul(bqs[:], bia[:, 0:1], SCALE)
            negs = cpool.tile([128, 1], F32)  # exp bias -SENT
            nc.vector.memset(negs[:], -SENT)
            ones_b = cpool.tile([1, 128], BF16)  # rank-1 bias lhsT
            nc.vector.memset(ones_b[:], 1.0)

            # ---- q/k projections, transposed: pT[d, r] = (x @ W_s.T).T ----
            def proj(s):
                p = pp.tile([128, RPC], F32, tag="proj")
                for ko in range(KO):
                    nc.tensor.matmul(
                        p[:],
                        lhsT=w[:, s, ko, :],
                        rhs=xT[:, ko, :],
                        start=(ko == 0),
                        stop=(ko == KO - 1),
                    )
                return p

            qT = cpool.tile([128, RPC], BF16)
            pq = proj(0)
            nc.scalar.activation(qT[:], pq[:], ACT.Identity, bias=bqs[:], scale=SCALE)
            kT = cpool.tile([128, RPC], BF16)
            pk = proj(1)
            nc.scalar.activation(kT[:], pk[:], ACT.Identity, bias=bia[:, 1:2])

            # ---- scores (transposed), bias+mask, exp ----
            ets = []
            for t in range(NT):
                g, jb = divmod(t, 2)
                sp = ps.tile([128, G], F32, tag="s")
                nc.tensor.matmul(
                    sp[:],
                    lhsT=kT[:, g * G + jb * 128: g * G + jb * 128 + 128],
                    rhs=qT[:, g * G:(g + 1) * G],
                    start=True,
                    stop=True,
                )
                a = wpool.tile([128, G], F32, tag="a")
                nc.vector.tensor_tensor(a[:], sp[:], bc[:, 0, t, :], ALU.add)
                nc.vector.tensor_tensor(a[:], a[:], bc[:, 1, t, :], ALU.add)
                # a += SENT * mask  (kept entries raised by SENT)
                nc.vector.scalar_tensor_tensor(
                    a[:], md[:, t, :], SENT, a[:], op0=ALU.mult, op1=ALU.add
                )
                # exp(a - SENT): kept -> exp(orig), masked -> exp(orig-100) ~ 0
                et = epool.tile([128, G], BF16, tag="et")
                nc.scalar.activation(et[:], a[:], ACT.Exp, bias=negs[:])
                ets.append(et)

            # ---- v in natural layout (bf16), ones column, rank-1 bias ----
            vna = cpool.tile([128, NT, VA], BF16)  # [j%128, j//128, d | 1]
            nc.vector.memset(vna[:, :, DQ:VA], 1.0)
            for jt in range(NT):
                pv = pvp.tile([128, DQ], F32, tag="vn")
                for ko in range(KO):
                    nc.tensor.matmul(
                        pv[:],
                        lhsT=xT[:, ko, jt * 128:(jt + 1) * 128],
                        rhs=w[:, 2, ko, :],
                        start=(ko == 0),
                        stop=False,
                    )
                nc.tensor.matmul(
                    pv[:], lhsT=ones_b[:], rhs=biar_b[:, 2, :],
                    start=False, stop=True,
                )
                nc.vector.tensor_copy(out=vna[:, jt, 0:DQ], in_=pv[:])

            # ---- PV matmul (+denominator via ones column), normalize ----
            out_sb = cpool.tile([128, NT, DQ], F32)
            out_r = out_d.rearrange("(t p) d -> p t d", p=128)
            for g in range(GPC):
                for rb in range(2):
                    t = 2 * g + rb
                    op = po.tile([128, VA], F32, tag="o")
                    for jb in range(2):
                        nc.tensor.matmul(
                            op[:],
                            lhsT=ets[2 * g + jb][:, rb * 128:(rb + 1) * 128],
                            rhs=vna[:, 2 * g + jb, :],
                            start=(jb == 0),
                            stop=(jb == 1),
                        )
                    rec = wpool.tile([128, 1], F32, tag="rec")
                    nc.vector.reciprocal(rec[:], op[:, DQ:VA])
                    nc.scalar.activation(
                        out_sb[:, t, :], op[:, 0:DQ], ACT.Copy, scale=rec[:]
                    )
                # per-graph output DMA so graph 0's store overlaps graph 1
                nc.sync.dma_start(
                    out_r[:, 2 * g:2 * g + 2, :], out_sb[:, 2 * g:2 * g + 2, :]
                )
    nc.compile()
    return nc


def get_nc() -> bass.Bass:
    if "nc" not in _CACHE:
        _CACHE["nc"] = build_nc()
    return _CACHE["nc"]


def make_in_maps(x, b, c, ptr, sparse_mask, Wq, bq, Wk, bk, Wv, bv):
    """Host-side sharding: slice the block-diagonal and transpose per layout."""
    x = np.asarray(x, dtype=np.float32)
    b = np.asarray(b, dtype=np.float32)
    c = np.asarray(c, dtype=np.float32)
    ptr = np.asarray(ptr)
    sparse_mask = np.asarray(sparse_mask, dtype=np.int32)
    w = np.ascontiguousarray(
        np.stack([np.asarray(Wq).T, np.asarray(Wk).T, np.asarray(Wv).T])
    ).astype(np.float32)  # [3, DIN, DQ]
    bias = np.ascontiguousarray(
        np.stack([np.asarray(bq), np.asarray(bk), np.asarray(bv)], axis=1)
    ).astype(np.float32)  # [DQ, 3]
    biasr = np.ascontiguousarray(
        np.stack([np.asarray(bq), np.asarray(bk), np.asarray(bv)], axis=0)[None]
    ).astype(np.float32)  # [1, 3, DQ]

    assert np.array_equal(
        np.asarray(ptr).ravel(), np.arange(NG + 1) * G
    ), "kernel compiled for uniform 256-node graphs"

    in_maps = []
    for i in range(NCORES):
        lo = i * RPC
        xT = np.ascontiguousarray(x[lo:lo + RPC].T)  # [DIN, RPC]
        bds, cds, mds = [], [], []
        for gl in range(GPC):
            blk = slice(lo + gl * G, lo + (gl + 1) * G)
            bds.append(b[blk, blk].T)
            cds.append(c[blk, blk].T)
            mds.append(sparse_mask[blk, blk].T)
        bcT = np.ascontiguousarray(
            np.stack([np.concatenate(bds, 0), np.concatenate(cds, 0)])
        ).astype(np.float32)  # [2, RPC, G]
        mdT = np.ascontiguousarray(np.concatenate(mds, 0)).astype(np.int32)
        in_maps.append(
            {"xT": xT, "w": w, "bias": bias, "biasr": biasr,
             "bcT": bcT, "mdT": mdT}
        )
    return in_maps


def run(inputs: dict, trace: bool = False):
    """Run on all 8 cores; returns (full_output, BassKernelResults)."""
    nc = get_nc()
    in_maps = make_in_maps(**inputs)
    res = run_bass_kernel_spmd(
        nc, in_maps, core_ids=list(range(NCORES)), trace=trace
    )
    out = np.concatenate([r["out"] for r in res.results], axis=0)
    return out.astype(np.float32), res


def kernel(**inputs) -> np.ndarray:
    out, _ = run(inputs, trace=False)
    return out
